# revision 28
# baseline (speedup 1.0000x reference)
"""Trainium2 Bass kernel v3 for nn_Discriminator: 5-layer GRU stack + projection.

Strategy
--------
Data parallel over batch (1024 -> 8 cores x 128) PLUS time-splitting within
each core: the 512-step scan is cut into TAU=8 segments of 64 steps. Each
segment re-converges from h=0 over WARM warmup steps (GRU state is
contractive). Segments run as 4 instruction streams (segment pairs (s, s+4)
batched into one 256-wide free dim), overlapping their serial chains.

v3 changes vs v2:
- The output projection y = W h4 + b is REMOVED from the serial loop.
  Layer-4 h slices accumulate in a 16-deep SBUF ring (the ring IS the
  state double-buffer), get flushed to a DRAM scratch H4T every 8
  windows, and a pipelined post-stage projects them to YT (matmul with
  ones-row bias + f32->f16 narrowing alternating ACT/DVE). This removes
  the per-window ymm PE pass, both ACT y-copies, and the y/PSUM
  R-quarter recycle coupling from the window chain.
- State tiles are per-PAIR rings [SA, 16, 512] instead of per-stream
  rotating tiles; h4 flush DMAs read 8 windows at once.
- DMA issue split across queues: x chunks alternate SP/GPSIMD, h4
  flushes on GPSIMD, post-stage DMAs on SP/Pool; ring-slot ones-row
  init is kept off the SP queue so window 0 isn't delayed.
- sigmoid/n-gate product/3-op update run per stream; tanh per PAIR
  (hardware A/B: pairing tanh won ~30-100us despite the sim preferring
  per-stream -- real per-call/sem overheads exceed the cost model's).
- Post-stage chunks are 4096 cols (full PSUM) because the HW tail does
  not pipeline across chunks (327us at CH=2048x32 vs ~55us modeled);
  fewer, bigger serial chains cost less.

Math (h-space, wavefront over layers), unchanged from v2:
- tick tau: layer l processes timestep tau-l; all layers' gates computed by
  shared matmuls over the packed state (rows: l0 0:32, l1 32:48, l2 48:56,
  l3 56:72, l4 72:104; row 104 = ones for biases).
- PSUM per stream: bank0 = R | HN, bank1 = Z | N. zc = 1-z via negated
  z-weights. update: h' = h + zc*(n - h) as three fp16 DVE ops.

All tensors fp16 except PSUM (f32). Input XT / output YT are fp16
[64, T(+4), 128] per core; host transposes and converts.
"""

import numpy as np

D = 64
T_FULL = 512
BZ = 1024
NCORES = 8
BC = BZ // NCORES  # 128
H = [32, 16, 8, 16, 32]
OFFS = [0, 32, 48, 56, 72]
SH = 104
SA = 105

TAU = 8          # time segments per core
WARM = 20        # warmup ticks per segment
SEG = T_FULL // TAU  # 64
NSTREAM = 4      # streams; stream s carries segments (s, s+4)
GC = 2           # chains (segments) per stream
FREE = GC * BC   # 256
PW = 2 * FREE    # pair width: 512
NWIN = WARM + SEG + 4  # 88 windows per stream (wavefront drains 4 ticks)
NWC = 11         # windows per x chunk (NWIN % NWC == 0)
NRING = 16       # state ring depth (also h4 flush batching x2)
NFL = 8          # windows per h4 flush
YW0 = WARM + 4   # first window whose post-state holds a valid h4 tick
CH = 4096        # post-stage columns per chunk (32 ticks x 128 batch)
NCHUNK = T_FULL * BC // CH  # 16
CHT = CH // BC   # 32 ticks per chunk
POST = True        # emit the y-projection post-stage (off: timing probe only)
YF32 = False       # (dead end: DMA cannot source PSUM in this API)
TM_POOL = False    # n-gate product on GPSIMD instead of DVE
SIG_PAIR = False   # sigmoid granularity: per pair vs per stream
TANH_PAIR = True   # tanh granularity: per pair vs per stream


def _build_weights(inp):
    """Pack reference GRU weights into h-space wavefront matrices (fp16)."""
    f32 = np.float32
    WR = np.zeros((SA, SH), f32)
    WZ = np.zeros((SA, SH), f32)
    WN = np.zeros((SA, SH), f32)
    WHN = np.zeros((SA, SH), f32)
    W0 = np.zeros((D, 3 * SH), f32)
    for l in range(5):
        dh, o = H[l], OFFS[l]
        w_ih = np.asarray(inp[f"w_ih_{l}"], f32)
        w_hh = np.asarray(inp[f"w_hh_{l}"], f32)
        b_ih = np.asarray(inp[f"b_ih_{l}"], f32)
        b_hh = np.asarray(inp[f"b_hh_{l}"], f32)
        Wir, Wiz, Win = w_ih[:dh], w_ih[dh:2 * dh], w_ih[2 * dh:]
        Whr, Whz, Whn = w_hh[:dh], w_hh[dh:2 * dh], w_hh[2 * dh:]
        bir, biz, bin_ = b_ih[:dh], b_ih[dh:2 * dh], b_ih[2 * dh:]
        bhr, bhz, bhn = b_hh[:dh], b_hh[dh:2 * dh], b_hh[2 * dh:]
        WR[o:o + dh, o:o + dh] = Whr.T
        WZ[o:o + dh, o:o + dh] = -Whz.T
        WHN[o:o + dh, o:o + dh] = Whn.T
        WR[SH, o:o + dh] = bir + bhr
        WZ[SH, o:o + dh] = -(biz + bhz)
        WN[SH, o:o + dh] = bin_
        WHN[SH, o:o + dh] = bhn
        if l == 0:
            W0[:, 0:32] = Wir.T
            W0[:, SH:SH + 32] = -Wiz.T
            W0[:, 2 * SH:2 * SH + 32] = Win.T
        else:
            po, pd = OFFS[l - 1], H[l - 1]
            WR[po:po + pd, o:o + dh] = Wir.T
            WZ[po:po + pd, o:o + dh] = -Wiz.T
            WN[po:po + pd, o:o + dh] = Win.T
    w_out = np.asarray(inp["w_out"], f32)
    b_out = np.asarray(inp["b_out"], f32)
    # post-stage projection weight: rows 0:32 = w_out.T, row 32 = bias
    WY2 = np.zeros((33, D), f32)
    WY2[0:32, :] = w_out.T
    WY2[32, :] = b_out

    f16 = np.float16
    IDENT = np.eye(SH, dtype=f16)
    ZINIT = np.zeros((SA, PW), f16)
    ZINIT[SH] = 1.0
    RSTZ = np.zeros((32, BC), f16)
    return {"WR": WR.astype(f16), "WZ": WZ.astype(f16), "WN": WN.astype(f16),
            "WHN": WHN.astype(f16), "W0": W0.astype(f16),
            "WY2": WY2.astype(f16), "ZINIT": ZINIT, "RSTZ": RSTZ,
            "IDENT": IDENT}


def _split_excess_waits(nc, limit=1):
    """The walrus build here accepts at most one sync-wait per instruction;
    Tile emits several on barrier drains etc. Split extras onto NoOps."""
    from concourse import mybir

    n_new = 0
    for f in nc.m.functions:
        for bb in f.blocks:
            changed = False
            new_list = []
            for ins in bb.instructions:
                si = ins.sync_info
                if si is not None and si.on_wait and len(si.on_wait) > limit:
                    waits = list(si.on_wait)
                    while len(waits) > limit:
                        chunk, waits = waits[:limit], waits[limit:]
                        nop = mybir.InstNoOp(
                            name=f"{ins.name}-ws{n_new}",
                            engine=ins.engine,
                            sync_info=mybir.SyncInfo(on_wait=chunk, on_update=[]),
                        )
                        new_list.append(nop)
                        n_new += 1
                    ins.sync_info = mybir.SyncInfo(
                        on_wait=list(waits), on_update=list(si.on_update)
                    )
                    changed = True
                new_list.append(ins)
            if changed:
                bb.instructions = new_list
    return n_new


_prog_cache = {}


def _build_program(T, reps=1):
    key = (T, reps)
    if key in _prog_cache:
        return _prog_cache[key]
    assert T == T_FULL, "kernel is specialized for T=512"
    import concourse.bass as bass
    import concourse.tile as tile
    from concourse import mybir

    f16 = mybir.dt.float16
    f32 = mybir.dt.float32
    SIG = mybir.ActivationFunctionType.Sigmoid
    TANH = mybir.ActivationFunctionType.Tanh
    COPY = mybir.ActivationFunctionType.Copy

    TP = T + 4  # XT padded with 4 zero ticks for wavefront drain

    vtag = f"{int(POST)}{int(TM_POOL)}{int(SIG_PAIR)}{int(TANH_PAIR)}{int(YF32)}"
    nc = bass.Bass(trn_type="TRN2", name=f"gru_v3_{T}_{reps}_{vtag}")
    XT = nc.dram_tensor("XT", [D, TP, BC], f16, kind="ExternalInput")
    dWR = nc.dram_tensor("WR", [SA, SH], f16, kind="ExternalInput")
    dWZ = nc.dram_tensor("WZ", [SA, SH], f16, kind="ExternalInput")
    dWN = nc.dram_tensor("WN", [SA, SH], f16, kind="ExternalInput")
    dWHN = nc.dram_tensor("WHN", [SA, SH], f16, kind="ExternalInput")
    dW0 = nc.dram_tensor("W0", [D, 3 * SH], f16, kind="ExternalInput")
    dWY2 = nc.dram_tensor("WY2", [33, D], f16, kind="ExternalInput")
    dZINIT = nc.dram_tensor("ZINIT", [SA, PW], f16, kind="ExternalInput")
    dIDENT = nc.dram_tensor("IDENT", [SH, SH], f16, kind="ExternalInput")
    dRSTZ = nc.dram_tensor("RSTZ", [32, BC], f16, kind="ExternalInput")
    H4T = nc.dram_tensor("H4T", [32, T, BC], f16, kind="Internal")
    YT = nc.dram_tensor("YT", [D, T, BC], f32 if YF32 else f16,
                        kind="ExternalOutput")

    with tile.TileContext(nc) as tc:
        with (
            tc.tile_pool(name="consts", bufs=1) as consts,
            tc.tile_pool(name="xpool", bufs=2 * NSTREAM) as xpool,
            tc.tile_pool(name="work", bufs=2 * NSTREAM) as work,
            tc.tile_pool(name="ypool", bufs=3) as ypool,
            tc.tile_pool(name="ps", bufs=1, space="PSUM") as ps,
        ):
            wr = consts.tile([SA, SH], f16, tag="wr")
            wz = consts.tile([SA, SH], f16, tag="wz")
            wn = consts.tile([SA, SH], f16, tag="wn")
            whn = consts.tile([SA, SH], f16, tag="whn")
            w0 = consts.tile([D, 3 * SH], f16, tag="w0")
            wy2 = consts.tile([33, D], f16, tag="wy2")
            ident = consts.tile([SH, SH], f16, tag="ident")
            for i, (sb, dr) in enumerate(((wr, dWR), (wz, dWZ), (wn, dWN),
                                          (whn, dWHN), (w0, dW0),
                                          (wy2, dWY2), (ident, dIDENT))):
                eng = nc.sync if i % 2 == 0 else nc.gpsimd
                eng.dma_start(out=sb[:], in_=dr[:])

            # state rings: one per pair, 16 deep, pair width 512
            rings = [consts.tile([SA, NRING, PW], f16, tag=f"ring{p}",
                                  name=f"ring{p}") for p in range(2)]
            # post-stage input staging (33rd row = ones for the bias)
            h4s = [consts.tile([33, CH], f16, tag=f"h4s{j}",
                                name=f"h4s{j}") for j in range(2)]

            for _rep in range(reps):
                # --- init: ones rows in every ring slot; zero state in the
                # slot read by window 0 (slot NRING-1) ---
                for p in range(2):
                    nc.sync.dma_start(out=rings[p][:, NRING - 1, :],
                                      in_=dZINIT[:])
                for p in range(2):
                    # ones rows for slots 0..14: only needed from window
                    # sl+1 onward; keep them off the SP queue so the first
                    # x chunks aren't delayed
                    for sl in range(NRING - 1):
                        nc.gpsimd.dma_start(
                            out=rings[p][SH:SA, sl, :],
                            in_=dZINIT[SH:SA, :])
                # ones row for h4s: ZINIT row SH is ones but only PW wide;
                # fill via CH/PW copies
                if _rep == 0:
                    for j in range(2):
                        for q in range(CH // PW):
                            nc.gpsimd.dma_start(
                                out=h4s[j][32:33, q * PW:(q + 1) * PW],
                                in_=dZINIT[SH:SA, :])

                pTv = ps.tile([SH, 4, 2, 2, 256], f32, tag="pT", name="pT")
                xcs = [None] * NSTREAM

                def tA0(s):  # chain A (segment s) tick at window 0
                    return SEG * s - WARM

                def tB0(s):  # chain B (segment s+4) tick at window 0
                    return SEG * (s + 4) - WARM

                def load_xchunk(s, w0_):
                    nw = min(NWC, NWIN - w0_)
                    xc = xpool.tile([D, NWC, FREE], f16, tag="xc", name="xc")
                    eng = nc.sync if s % 2 == 0 else nc.gpsimd
                    for half, t0 in ((0, tA0(s) + w0_), (1, tB0(s) + w0_)):
                        lo = half * BC
                        if t0 >= 0:
                            eng.dma_start(
                                out=xc[:, 0:nw, lo:lo + BC],
                                in_=XT[:, t0:t0 + nw, :])
                        elif t0 + nw > 0:
                            k = -t0
                            eng.dma_start(
                                out=xc[:, k:nw, lo:lo + BC],
                                in_=XT[:, 0:nw - k, :])
                            eng.dma_start(
                                out=xc[:, 0:k, lo:lo + BC],
                                in_=XT[:, 0:k, :])  # garbage warmup pad
                        else:
                            eng.dma_start(
                                out=xc[:, 0:nw, lo:lo + BC],
                                in_=XT[:, 0:nw, :])  # garbage warmup pad
                    xcs[s] = xc

                def emit_xmm_rz(w):
                    """x prefetch matmuls for window w's R/Z quarters."""
                    for s in range(NSTREAM):
                        xw = xcs[s][:, w % NWC, :]
                        nc.tensor.matmul(pTv[:, s, 0, 0, :], w0[:, 0:SH],
                                         xw, start=True, stop=False)
                        nc.tensor.matmul(pTv[:, s, 1, 0, :],
                                         w0[:, SH:2 * SH], xw,
                                         start=True, stop=False)

                def window_all(w):
                    """One window for all streams. Chain-critical ops run per
                    stream; emission order matches expected readiness (each
                    engine executes its queue in order)."""
                    sl = w % NRING
                    pv = (w - 1) % NRING
                    xws, rzs, ntp = {}, {}, {}
                    if w % NWC == 0:
                        for s in range(NSTREAM):
                            load_xchunk(s, w)
                    for s in range(NSTREAM):
                        xws[s] = xcs[s][:, w % NWC, :]
                    if w % NWC == 0:
                        # chunk-boundary window: x matmuls were not
                        # prefetched at the previous window's tail
                        emit_xmm_rz(w)

                    def prev_ap(s):
                        p, js = s // 2, s % 2
                        return rings[p][:, pv, js * FREE:(js + 1) * FREE]

                    # PSUM per stream: bank0 = R | HN, bank1 = Z | N; group
                    # order per quarter: R: xmmR,mmR ; HN: mmHN ;
                    # Z: xmmZ,mmZ ; N: xmmN,mmN,fold
                    for s in range(NSTREAM):
                        prev = prev_ap(s)
                        nc.tensor.matmul(pTv[:, s, 0, 0, :], wr[:],
                                         prev, start=False, stop=True)
                        nc.tensor.matmul(pTv[:, s, 1, 0, :], wz[:],
                                         prev, start=False, stop=True)
                        nc.tensor.matmul(pTv[:, s, 0, 1, :], whn[:],
                                         prev, start=True, stop=True)
                    if SIG_PAIR:
                        for pair in range(2):
                            rz = work.tile([SH, 2, 2, 256], f16, tag="rz",
                                           name="rz")
                            nc.scalar.activation(
                                rz[:], pTv[:, 2 * pair:2 * pair + 2, :, 0, :],
                                SIG)
                            rzs[2 * pair] = rz[:, 0, :, :]
                            rzs[2 * pair + 1] = rz[:, 1, :, :]
                    else:
                        for s in range(NSTREAM):
                            rz = work.tile([SH, 2, 256], f16, tag="rz",
                                           name="rz")
                            nc.scalar.activation(rz[:], pTv[:, s, :, 0, :],
                                                 SIG)
                            rzs[s] = rz
                    # N-gate input matmuls
                    for s in range(NSTREAM):
                        nc.tensor.matmul(pTv[:, s, 1, 1, :],
                                         w0[:, 2 * SH:3 * SH], xws[s],
                                         start=True, stop=False)
                        nc.tensor.matmul(pTv[:, s, 1, 1, :], wn[:],
                                         prev_ap(s), start=False,
                                         stop=False)
                    # DVE: per-stream n-gate product; PE folds into pN
                    for s in range(NSTREAM):
                        tm = work.tile([SH, FREE], f16, tag="tm", name="tm")
                        eng = nc.gpsimd if TM_POOL else nc.vector
                        eng.tensor_mul(tm[:], rzs[s][:, 0, :],
                                       pTv[:, s, 0, 1, :])
                        nc.tensor.matmul(pTv[:, s, 1, 1, :], ident[:],
                                         tm[:], start=False, stop=True)
                    if TANH_PAIR:
                        for pair in range(2):
                            nt = work.tile([SH, 2, 256], f16, tag="nt",
                                           name="nt")
                            nc.scalar.activation(
                                nt[:],
                                pTv[:, 2 * pair:2 * pair + 2, 1, 1, :],
                                TANH)
                            ntp[2 * pair] = nt[:, 0, :]
                            ntp[2 * pair + 1] = nt[:, 1, :]
                    else:
                        for s in range(NSTREAM):
                            nt = work.tile([SH, FREE], f16, tag="nt",
                                           name="nt")
                            nc.scalar.activation(nt[:], pTv[:, s, 1, 1, :],
                                                 TANH)
                            ntp[s] = nt[:]
                    # per-stream state update into ring slot sl
                    for s in range(NSTREAM):
                        pair, js = s // 2, s % 2
                        prev = prev_ap(s)
                        zc_s = rzs[s][:, 1, :]
                        d = work.tile([SH, FREE], f16, tag="d", name="d")
                        nc.vector.tensor_sub(d[:], ntp[s], prev[0:SH, :])
                        p = work.tile([SH, FREE], f16, tag="p", name="p")
                        nc.vector.tensor_mul(p[:], d[:], zc_s)
                        gdst = rings[pair][0:SH, sl, js * FREE:(js + 1) * FREE]
                        nc.vector.tensor_add(gdst, p[:], prev[0:SH, :])
                        if s == 0:
                            # segment-0 warmup resets (chain A of stream 0)
                            if w == WARM - 1:
                                nc.sync.dma_start(
                                    out=rings[0][0:SH, sl, 0:BC],
                                    in_=dZINIT[0:SH, 0:BC])
                            elif WARM <= w < WARM + 4:
                                l = w - WARM + 1
                                nc.sync.dma_start(
                                    out=rings[0][OFFS[l]:OFFS[l] + H[l],
                                                 sl, 0:BC],
                                    in_=dRSTZ[0:H[l], :])
                    # prefetch next window's x matmuls (quarters are free
                    # once this window's sigmoid has read them); skip at
                    # chunk boundaries where the next chunk isn't loaded yet
                    if w + 1 < NWIN and (w + 1) % NWC != 0:
                        emit_xmm_rz(w + 1)
                    # h4 flush every NFL windows once ticks are valid
                    if w >= YW0 + NFL - 1 and (w - YW0) % NFL == NFL - 1:
                        k0 = w - YW0 - NFL + 1  # first tick of this flush
                        s0 = (w - NFL + 1) % NRING  # first ring slot
                        assert s0 + NFL <= NRING
                        for s in range(NSTREAM):
                            pair, js = s // 2, s % 2
                            for half, seg in ((0, s), (1, s + 4)):
                                t0 = SEG * seg + k0
                                lo = js * FREE + half * BC
                                nc.gpsimd.dma_start(
                                    out=H4T[:, t0:t0 + NFL, :],
                                    in_=rings[pair][OFFS[4]:OFFS[4] + 32,
                                                    s0:s0 + NFL,
                                                    lo:lo + BC])

                for w in range(NWIN):
                    window_all(w)

                # --- post-stage: y = WY2^T @ [h4; 1] over 16 chunks of
                # 4096 cols; the whole PSUM is one chunk, the f32->f16
                # narrow runs split across ACT and DVE in parallel ---
                for c in range(NCHUNK if POST else 0):
                    t0 = c * CHT
                    hb = h4s[c % 2]
                    nc.sync.dma_start(out=hb[0:32, :],
                                      in_=H4T[:, t0:t0 + CHT, :])
                    yo = ypool.tile([D, CH], f16, tag="yo", name="yo")
                    HT = CHT // 2
                    # half A: banks 0-3 -> ACT narrow -> scalar-queue DMA,
                    # half B: banks 4-7 -> DVE narrow -> vector-queue DMA.
                    # Each half's chain stays on one engine queue (no
                    # cross-queue sem hops in the tail-critical path).
                    for q in range(4):
                        nc.tensor.matmul(
                            pTv[0:D, q // 2, q % 2, :, :],
                            wy2[:], hb[:, q * 512:(q + 1) * 512],
                            start=True, stop=True)
                    nc.scalar.activation(yo[:, 0:CH // 2],
                                         pTv[0:D, 0:2, :, :, :], COPY)
                    nc.scalar.dma_start(
                        out=YT[:, t0:t0 + HT, :],
                        in_=yo[:, 0:CH // 2]
                            .rearrange("p (t b) -> p t b", t=HT))
                    for q in range(4, 8):
                        nc.tensor.matmul(
                            pTv[0:D, q // 2, q % 2, :, :],
                            wy2[:], hb[:, q * 512:(q + 1) * 512],
                            start=True, stop=True)
                    nc.vector.tensor_scalar_mul(yo[:, CH // 2:CH],
                                                pTv[0:D, 2:4, :, :, :], 1.0)
                    nc.sync.dma_start(
                        out=YT[:, t0 + HT:t0 + CHT, :],
                        in_=yo[:, CH // 2:CH]
                            .rearrange("p (t b) -> p t b", t=HT))

    _split_excess_waits(nc)
    _prog_cache[key] = nc
    return nc


def _prep_inputs(X_full, weights, T):
    """X_full [BZ, T, D] fp32 -> per-core in_maps with fp16 padded XT."""
    maps = []
    for c in range(NCORES):
        xs = X_full[c * BC:(c + 1) * BC]  # [BC, T, D]
        xt = np.zeros((D, T + 4, BC), np.float16)
        xt[:, :T, :] = xs.transpose(2, 1, 0).astype(np.float16)
        maps.append({"XT": xt, **weights})
    return maps


def _run(X_full, weights, T):
    from concourse.bass_utils import run_bass_kernel_spmd

    nc = _build_program(T)
    in_maps = _prep_inputs(X_full, weights, T)
    res = run_bass_kernel_spmd(nc, in_maps, core_ids=list(range(NCORES)))
    outs = []
    for c in range(NCORES):
        YTc = res.results[c]["YT"]  # [D, T, BC] fp16
        outs.append(np.ascontiguousarray(
            YTc.astype(np.float32).transpose(2, 1, 0)))
    return np.concatenate(outs, 0)


def kernel(**inputs):
    X = np.asarray(inputs["imputed_X"], np.float32)
    weights = _build_weights(inputs)
    return _run(X, weights, X.shape[1])


# revision 34
# speedup vs baseline: 1.0090x; 1.0090x over previous
"""Trainium2 Bass kernel v3 for nn_Discriminator: 5-layer GRU stack + projection.

Strategy
--------
Data parallel over batch (1024 -> 8 cores x 128) PLUS time-splitting within
each core: the 512-step scan is cut into TAU=8 segments of 64 steps. Each
segment re-converges from h=0 over WARM warmup steps (GRU state is
contractive). Segments run as 4 instruction streams (segment pairs (s, s+4)
batched into one 256-wide free dim), overlapping their serial chains.

v3 changes vs v2:
- The output projection y = W h4 + b is REMOVED from the serial loop.
  Layer-4 h slices accumulate in a 16-deep SBUF ring (the ring IS the
  state double-buffer), get flushed to a DRAM scratch H4T every 8
  windows, and a pipelined post-stage projects them to YT (matmul with
  ones-row bias + f32->f16 narrowing alternating ACT/DVE). This removes
  the per-window ymm PE pass, both ACT y-copies, and the y/PSUM
  R-quarter recycle coupling from the window chain.
- State tiles are per-PAIR rings [SA, 16, 512] instead of per-stream
  rotating tiles; h4 flush DMAs read 8 windows at once.
- DMA issue split across queues: x chunks alternate SP/GPSIMD, h4
  flushes on GPSIMD, post-stage DMAs on SP/Pool; ring-slot ones-row
  init is kept off the SP queue so window 0 isn't delayed.
- sigmoid/n-gate product/3-op update run per stream; tanh per PAIR
  (hardware A/B: pairing tanh won ~30-100us despite the sim preferring
  per-stream -- real per-call/sem overheads exceed the cost model's).
- Post-stage chunks are 4096 cols (full PSUM) because the HW tail does
  not pipeline across chunks (327us at CH=2048x32 vs ~55us modeled);
  fewer, bigger serial chains cost less.

Math (h-space, wavefront over layers), unchanged from v2:
- tick tau: layer l processes timestep tau-l; all layers' gates computed by
  shared matmuls over the packed state (rows: l0 0:32, l1 32:48, l2 48:56,
  l3 56:72, l4 72:104; row 104 = ones for biases).
- PSUM per stream: bank0 = R | HN, bank1 = Z | N. zc = 1-z via negated
  z-weights. update: h' = h + zc*(n - h) as three fp16 DVE ops.

All tensors fp16 except PSUM (f32). Input XT / output YT are fp16
[64, T(+4), 128] per core; host transposes and converts.
"""

import numpy as np

D = 64
T_FULL = 512
BZ = 1024
NCORES = 8
BC = BZ // NCORES  # 128
H = [32, 16, 8, 16, 32]
OFFS = [0, 32, 48, 56, 72]
SH = 104
SA = 105

TAU = 8          # time segments per core
WARM = 20        # warmup ticks per segment
SEG = T_FULL // TAU  # 64
NSTREAM = 4      # streams; stream s carries segments (s, s+4)
GC = 2           # chains (segments) per stream
FREE = GC * BC   # 256
PW = 2 * FREE    # pair width: 512
NWIN = WARM + SEG + 4  # 88 windows per stream (wavefront drains 4 ticks)
NWC = 11         # windows per x chunk (NWIN % NWC == 0)
NRING = 64       # ring holds the FULL valid h4 history in SBUF
NFL = 8          # windows per h4 flush
YW0 = WARM + 4   # first window whose post-state holds a valid h4 tick
CH = 4096        # post-stage columns per chunk (32 ticks x 128 batch)
NCHUNK = T_FULL * BC // CH  # 16
CHT = CH // BC   # 32 ticks per chunk
POST = True        # emit the y-projection post-stage (off: timing probe only)
YF32 = False       # (dead end: DMA cannot source PSUM in this API)
TM_POOL = False    # n-gate product on GPSIMD instead of DVE
SIG_PAIR = False   # sigmoid granularity: per pair vs per stream
TANH_PAIR = True   # tanh granularity: per pair vs per stream


def _build_weights(inp):
    """Pack reference GRU weights into h-space wavefront matrices (fp16)."""
    f32 = np.float32
    WR = np.zeros((SA, SH), f32)
    WZ = np.zeros((SA, SH), f32)
    WN = np.zeros((SA, SH), f32)
    WHN = np.zeros((SA, SH), f32)
    W0 = np.zeros((D, 3 * SH), f32)
    for l in range(5):
        dh, o = H[l], OFFS[l]
        w_ih = np.asarray(inp[f"w_ih_{l}"], f32)
        w_hh = np.asarray(inp[f"w_hh_{l}"], f32)
        b_ih = np.asarray(inp[f"b_ih_{l}"], f32)
        b_hh = np.asarray(inp[f"b_hh_{l}"], f32)
        Wir, Wiz, Win = w_ih[:dh], w_ih[dh:2 * dh], w_ih[2 * dh:]
        Whr, Whz, Whn = w_hh[:dh], w_hh[dh:2 * dh], w_hh[2 * dh:]
        bir, biz, bin_ = b_ih[:dh], b_ih[dh:2 * dh], b_ih[2 * dh:]
        bhr, bhz, bhn = b_hh[:dh], b_hh[dh:2 * dh], b_hh[2 * dh:]
        WR[o:o + dh, o:o + dh] = Whr.T
        WZ[o:o + dh, o:o + dh] = -Whz.T
        WHN[o:o + dh, o:o + dh] = Whn.T
        WR[SH, o:o + dh] = bir + bhr
        WZ[SH, o:o + dh] = -(biz + bhz)
        WN[SH, o:o + dh] = bin_
        WHN[SH, o:o + dh] = bhn
        if l == 0:
            W0[:, 0:32] = Wir.T
            W0[:, SH:SH + 32] = -Wiz.T
            W0[:, 2 * SH:2 * SH + 32] = Win.T
        else:
            po, pd = OFFS[l - 1], H[l - 1]
            WR[po:po + pd, o:o + dh] = Wir.T
            WZ[po:po + pd, o:o + dh] = -Wiz.T
            WN[po:po + pd, o:o + dh] = Win.T
    w_out = np.asarray(inp["w_out"], f32)
    b_out = np.asarray(inp["b_out"], f32)
    # post-stage projection weight, full state height so a [64:105] slice
    # (legal matmul base partition) pairs with the same ring slice:
    # rows 72:104 = w_out.T (l4 = h4), row 104 = bias (ones row)
    WY2 = np.zeros((SA, D), f32)
    WY2[OFFS[4]:OFFS[4] + 32, :] = w_out.T
    WY2[SH, :] = b_out

    f16 = np.float16
    IDENT = np.eye(SH, dtype=f16)
    ZINIT = np.zeros((SA, PW), f16)
    ZINIT[SH] = 1.0
    RSTZ = np.zeros((32, BC), f16)
    return {"WR": WR.astype(f16), "WZ": WZ.astype(f16), "WN": WN.astype(f16),
            "WHN": WHN.astype(f16), "W0": W0.astype(f16),
            "WY2": WY2.astype(f16), "ZINIT": ZINIT, "RSTZ": RSTZ,
            "IDENT": IDENT}


def _split_excess_waits(nc, limit=1):
    """The walrus build here accepts at most one sync-wait per instruction;
    Tile emits several on barrier drains etc. Split extras onto NoOps."""
    from concourse import mybir

    n_new = 0
    for f in nc.m.functions:
        for bb in f.blocks:
            changed = False
            new_list = []
            for ins in bb.instructions:
                si = ins.sync_info
                if si is not None and si.on_wait and len(si.on_wait) > limit:
                    waits = list(si.on_wait)
                    while len(waits) > limit:
                        chunk, waits = waits[:limit], waits[limit:]
                        nop = mybir.InstNoOp(
                            name=f"{ins.name}-ws{n_new}",
                            engine=ins.engine,
                            sync_info=mybir.SyncInfo(on_wait=chunk, on_update=[]),
                        )
                        new_list.append(nop)
                        n_new += 1
                    ins.sync_info = mybir.SyncInfo(
                        on_wait=list(waits), on_update=list(si.on_update)
                    )
                    changed = True
                new_list.append(ins)
            if changed:
                bb.instructions = new_list
    return n_new


_prog_cache = {}


def _build_program(T, reps=1):
    key = (T, reps)
    if key in _prog_cache:
        return _prog_cache[key]
    assert T == T_FULL, "kernel is specialized for T=512"
    import concourse.bass as bass
    import concourse.tile as tile
    from concourse import mybir

    f16 = mybir.dt.float16
    f32 = mybir.dt.float32
    SIG = mybir.ActivationFunctionType.Sigmoid
    TANH = mybir.ActivationFunctionType.Tanh
    COPY = mybir.ActivationFunctionType.Copy

    TP = T + 4  # XT padded with 4 zero ticks for wavefront drain

    vtag = f"{int(POST)}{int(TM_POOL)}{int(SIG_PAIR)}{int(TANH_PAIR)}{int(YF32)}"
    nc = bass.Bass(trn_type="TRN2", name=f"gru_v3_{T}_{reps}_{vtag}")
    XT = nc.dram_tensor("XT", [D, TP, BC], f16, kind="ExternalInput")
    dWR = nc.dram_tensor("WR", [SA, SH], f16, kind="ExternalInput")
    dWZ = nc.dram_tensor("WZ", [SA, SH], f16, kind="ExternalInput")
    dWN = nc.dram_tensor("WN", [SA, SH], f16, kind="ExternalInput")
    dWHN = nc.dram_tensor("WHN", [SA, SH], f16, kind="ExternalInput")
    dW0 = nc.dram_tensor("W0", [D, 3 * SH], f16, kind="ExternalInput")
    dWY2 = nc.dram_tensor("WY2", [SA, D], f16, kind="ExternalInput")
    dZINIT = nc.dram_tensor("ZINIT", [SA, PW], f16, kind="ExternalInput")
    dIDENT = nc.dram_tensor("IDENT", [SH, SH], f16, kind="ExternalInput")
    dRSTZ = nc.dram_tensor("RSTZ", [32, BC], f16, kind="ExternalInput")
    YT = nc.dram_tensor("YT", [D, T, BC], f32 if YF32 else f16,
                        kind="ExternalOutput")

    with tile.TileContext(nc) as tc:
        with (
            tc.tile_pool(name="consts", bufs=1) as consts,
            tc.tile_pool(name="xpool", bufs=2 * NSTREAM) as xpool,
            tc.tile_pool(name="work", bufs=5) as work,
            tc.tile_pool(name="ypool", bufs=1) as ypool,
            tc.tile_pool(name="ps", bufs=1, space="PSUM") as ps,
        ):
            wr = consts.tile([SA, SH], f16, tag="wr")
            wz = consts.tile([SA, SH], f16, tag="wz")
            wn = consts.tile([SA, SH], f16, tag="wn")
            whn = consts.tile([SA, SH], f16, tag="whn")
            w0 = consts.tile([D, 3 * SH], f16, tag="w0")
            wy2 = consts.tile([SA, D], f16, tag="wy2")
            ident = consts.tile([SH, SH], f16, tag="ident")
            for i, (sb, dr) in enumerate(((wr, dWR), (wz, dWZ), (wn, dWN),
                                          (whn, dWHN), (w0, dW0),
                                          (wy2, dWY2), (ident, dIDENT))):
                eng = nc.sync if i % 2 == 0 else nc.gpsimd
                eng.dma_start(out=sb[:], in_=dr[:])

            # state rings: one per pair, 16 deep, pair width 512
            rings = [consts.tile([SA, NRING, PW], f16, tag=f"ring{p}",
                                  name=f"ring{p}") for p in range(2)]

            for _rep in range(reps):
                # --- init: ones rows in every ring slot; zero state in the
                # slot read by window 0 (slot NRING-1) ---
                for p in range(2):
                    nc.sync.dma_start(out=rings[p][:, NRING - 1, :],
                                      in_=dZINIT[:])
                for p in range(2):
                    # ones rows: slot k is read from window k+1 on, so the
                    # first few get individual early DMAs and the rest one
                    # bulk memset (done by ~25us, needed by window 9+)
                    for sl in range(NRING - 1):
                        nc.gpsimd.dma_start(
                            out=rings[p][SH:SA, sl, :],
                            in_=dZINIT[SH:SA, :])

                pTv = ps.tile([SH, 4, 2, 2, 256], f32, tag="pT", name="pT")
                xcs = [None] * NSTREAM

                def tA0(s):  # chain A (segment s) tick at window 0
                    return SEG * s - WARM

                def tB0(s):  # chain B (segment s+4) tick at window 0
                    return SEG * (s + 4) - WARM

                def load_xchunk(s, w0_):
                    nw = min(NWC, NWIN - w0_)
                    xc = xpool.tile([D, NWC, FREE], f16, tag="xc", name="xc")
                    eng = nc.sync if s % 2 == 0 else nc.gpsimd
                    for half, t0 in ((0, tA0(s) + w0_), (1, tB0(s) + w0_)):
                        lo = half * BC
                        if t0 >= 0:
                            eng.dma_start(
                                out=xc[:, 0:nw, lo:lo + BC],
                                in_=XT[:, t0:t0 + nw, :])
                        elif t0 + nw > 0:
                            k = -t0
                            eng.dma_start(
                                out=xc[:, k:nw, lo:lo + BC],
                                in_=XT[:, 0:nw - k, :])
                            eng.dma_start(
                                out=xc[:, 0:k, lo:lo + BC],
                                in_=XT[:, 0:k, :])  # garbage warmup pad
                        else:
                            eng.dma_start(
                                out=xc[:, 0:nw, lo:lo + BC],
                                in_=XT[:, 0:nw, :])  # garbage warmup pad
                    xcs[s] = xc

                def emit_xmm_rz(w):
                    """x prefetch matmuls for window w's R/Z quarters."""
                    for s in range(NSTREAM):
                        xw = xcs[s][:, w % NWC, :]
                        nc.tensor.matmul(pTv[:, s, 0, 0, :], w0[:, 0:SH],
                                         xw, start=True, stop=False)
                        nc.tensor.matmul(pTv[:, s, 1, 0, :],
                                         w0[:, SH:2 * SH], xw,
                                         start=True, stop=False)

                def window_all(w):
                    """One window for all streams. Chain-critical ops run per
                    stream; emission order matches expected readiness (each
                    engine executes its queue in order)."""
                    sl = w % NRING
                    pv = (w - 1) % NRING
                    xws, rzs, ntp = {}, {}, {}
                    if w % NWC == 0:
                        for s in range(NSTREAM):
                            load_xchunk(s, w)
                    for s in range(NSTREAM):
                        xws[s] = xcs[s][:, w % NWC, :]
                    if w % NWC == 0:
                        # chunk-boundary window: x matmuls were not
                        # prefetched at the previous window's tail
                        emit_xmm_rz(w)

                    def prev_ap(s):
                        p, js = s // 2, s % 2
                        return rings[p][:, pv, js * FREE:(js + 1) * FREE]

                    # PSUM per stream: bank0 = R | HN, bank1 = Z | N; group
                    # order per quarter: R: xmmR,mmR ; HN: mmHN ;
                    # Z: xmmZ,mmZ ; N: xmmN,mmN,fold
                    for s in range(NSTREAM):
                        prev = prev_ap(s)
                        nc.tensor.matmul(pTv[:, s, 0, 0, :], wr[:],
                                         prev, start=False, stop=True)
                        nc.tensor.matmul(pTv[:, s, 1, 0, :], wz[:],
                                         prev, start=False, stop=True)
                        nc.tensor.matmul(pTv[:, s, 0, 1, :], whn[:],
                                         prev, start=True, stop=True)
                    if SIG_PAIR:
                        for pair in range(2):
                            rz = work.tile([SH, 2, 2, 256], f16, tag="rz",
                                           name="rz")
                            nc.scalar.activation(
                                rz[:], pTv[:, 2 * pair:2 * pair + 2, :, 0, :],
                                SIG)
                            rzs[2 * pair] = rz[:, 0, :, :]
                            rzs[2 * pair + 1] = rz[:, 1, :, :]
                    else:
                        for s in range(NSTREAM):
                            rz = work.tile([SH, 2, 256], f16, tag="rz",
                                           name="rz")
                            nc.scalar.activation(rz[:], pTv[:, s, :, 0, :],
                                                 SIG)
                            rzs[s] = rz
                    # N-gate input matmuls
                    for s in range(NSTREAM):
                        nc.tensor.matmul(pTv[:, s, 1, 1, :],
                                         w0[:, 2 * SH:3 * SH], xws[s],
                                         start=True, stop=False)
                        nc.tensor.matmul(pTv[:, s, 1, 1, :], wn[:],
                                         prev_ap(s), start=False,
                                         stop=False)
                    # DVE: per-stream n-gate product; PE folds into pN
                    for s in range(NSTREAM):
                        tm = work.tile([SH, FREE], f16, tag="tm", name="tm")
                        eng = nc.gpsimd if TM_POOL else nc.vector
                        eng.tensor_mul(tm[:], rzs[s][:, 0, :],
                                       pTv[:, s, 0, 1, :])
                        nc.tensor.matmul(pTv[:, s, 1, 1, :], ident[:],
                                         tm[:], start=False, stop=True)
                    if TANH_PAIR:
                        for pair in range(2):
                            nt = work.tile([SH, 2, 256], f16, tag="nt",
                                           name="nt")
                            nc.scalar.activation(
                                nt[:],
                                pTv[:, 2 * pair:2 * pair + 2, 1, 1, :],
                                TANH)
                            ntp[2 * pair] = nt[:, 0, :]
                            ntp[2 * pair + 1] = nt[:, 1, :]
                    else:
                        for s in range(NSTREAM):
                            nt = work.tile([SH, FREE], f16, tag="nt",
                                           name="nt")
                            nc.scalar.activation(nt[:], pTv[:, s, 1, 1, :],
                                                 TANH)
                            ntp[s] = nt[:]
                    # per-stream state update into ring slot sl
                    for s in range(NSTREAM):
                        pair, js = s // 2, s % 2
                        prev = prev_ap(s)
                        zc_s = rzs[s][:, 1, :]
                        d = work.tile([SH, FREE], f16, tag="d", name="d")
                        nc.vector.tensor_sub(d[:], ntp[s], prev[0:SH, :])
                        p = work.tile([SH, FREE], f16, tag="p", name="p")
                        nc.vector.tensor_mul(p[:], d[:], zc_s)
                        gdst = rings[pair][0:SH, sl, js * FREE:(js + 1) * FREE]
                        nc.vector.tensor_add(gdst, p[:], prev[0:SH, :])
                        if s == 0:
                            # segment-0 warmup resets (chain A of stream 0)
                            if w == WARM - 1:
                                nc.sync.dma_start(
                                    out=rings[0][0:SH, sl, 0:BC],
                                    in_=dZINIT[0:SH, 0:BC])
                            elif WARM <= w < WARM + 4:
                                l = w - WARM + 1
                                nc.sync.dma_start(
                                    out=rings[0][OFFS[l]:OFFS[l] + H[l],
                                                 sl, 0:BC],
                                    in_=dRSTZ[0:H[l], :])
                    # prefetch next window's x matmuls (quarters are free
                    # once this window's sigmoid has read them); skip at
                    # chunk boundaries where the next chunk isn't loaded yet
                    if w + 1 < NWIN and (w + 1) % NWC != 0:
                        emit_xmm_rz(w + 1)

                for w in range(NWIN):
                    window_all(w)

                # --- post-stage: y = WY2^T @ [h4; 1] over 16 chunks of
                # 4096 cols (32 ticks x 128 batch of one chain), matmuls
                # reading the state ring DIRECTLY (rows 72:105 = h4 + the
                # ones row); no DRAM round trip. Window w's h4 tick is
                # w - YW0, stored in ring slot w % NRING. ---
                for c in range(NCHUNK if POST else 0):
                    seg, khalf = divmod(c, 2)
                    k0 = 32 * khalf
                    t0 = SEG * seg + k0
                    st_ = seg % 4
                    pair, js = st_ // 2, st_ % 2
                    half = seg // 4
                    lo = js * FREE + half * BC
                    for q in range(8):
                        s0 = (YW0 + k0 + 4 * q) % NRING
                        nc.tensor.matmul(
                            pTv[0:D, q // 2, q % 2, :, :],
                            wy2[64:SA, :],
                            rings[pair][64:SA, s0:s0 + 4, lo:lo + BC],
                            start=True, stop=True)
                    yo = ypool.tile([D, CH], f16, tag="yo", name="yo")
                    nc.scalar.activation(yo[:, 0:CH // 2],
                                         pTv[0:D, 0:2, :, :, :], COPY)
                    nc.vector.tensor_scalar_mul(yo[:, CH // 2:CH],
                                                pTv[0:D, 2:4, :, :, :], 1.0)
                    nc.gpsimd.dma_start(
                        out=YT[:, t0:t0 + CHT, :],
                        in_=yo[:].rearrange("p (t b) -> p t b", t=CHT))

    _split_excess_waits(nc)
    _prog_cache[key] = nc
    return nc


def _prep_inputs(X_full, weights, T):
    """X_full [BZ, T, D] fp32 -> per-core in_maps with fp16 padded XT."""
    maps = []
    for c in range(NCORES):
        xs = X_full[c * BC:(c + 1) * BC]  # [BC, T, D]
        xt = np.zeros((D, T + 4, BC), np.float16)
        xt[:, :T, :] = xs.transpose(2, 1, 0).astype(np.float16)
        maps.append({"XT": xt, **weights})
    return maps


def _run(X_full, weights, T):
    from concourse.bass_utils import run_bass_kernel_spmd

    nc = _build_program(T)
    in_maps = _prep_inputs(X_full, weights, T)
    res = run_bass_kernel_spmd(nc, in_maps, core_ids=list(range(NCORES)))
    outs = []
    for c in range(NCORES):
        YTc = res.results[c]["YT"]  # [D, T, BC] fp16
        outs.append(np.ascontiguousarray(
            YTc.astype(np.float32).transpose(2, 1, 0)))
    return np.concatenate(outs, 0)


def kernel(**inputs):
    X = np.asarray(inputs["imputed_X"], np.float32)
    weights = _build_weights(inputs)
    return _run(X, weights, X.shape[1])


# revision 36
# speedup vs baseline: 1.0423x; 1.0331x over previous
"""Trainium2 Bass kernel v3 for nn_Discriminator: 5-layer GRU stack + projection.

Strategy
--------
Data parallel over batch (1024 -> 8 cores x 128) PLUS time-splitting within
each core: the 512-step scan is cut into TAU=8 segments of 64 steps. Each
segment re-converges from h=0 over WARM warmup steps (GRU state is
contractive). Segments run as 4 instruction streams (segment pairs (s, s+4)
batched into one 256-wide free dim), overlapping their serial chains.

v3 changes vs v2:
- The output projection y = W h4 + b is REMOVED from the serial loop.
  Layer-4 h slices accumulate in a 16-deep SBUF ring (the ring IS the
  state double-buffer), get flushed to a DRAM scratch H4T every 8
  windows, and a pipelined post-stage projects them to YT (matmul with
  ones-row bias + f32->f16 narrowing alternating ACT/DVE). This removes
  the per-window ymm PE pass, both ACT y-copies, and the y/PSUM
  R-quarter recycle coupling from the window chain.
- State tiles are per-PAIR rings [SA, 16, 512] instead of per-stream
  rotating tiles; h4 flush DMAs read 8 windows at once.
- DMA issue split across queues: x chunks alternate SP/GPSIMD, h4
  flushes on GPSIMD, post-stage DMAs on SP/Pool; ring-slot ones-row
  init is kept off the SP queue so window 0 isn't delayed.
- sigmoid/n-gate product/3-op update run per stream; tanh per PAIR
  (hardware A/B: pairing tanh won ~30-100us despite the sim preferring
  per-stream -- real per-call/sem overheads exceed the cost model's).
- Post-stage chunks are 4096 cols (full PSUM) because the HW tail does
  not pipeline across chunks (327us at CH=2048x32 vs ~55us modeled);
  fewer, bigger serial chains cost less.

Math (h-space, wavefront over layers), unchanged from v2:
- tick tau: layer l processes timestep tau-l; all layers' gates computed by
  shared matmuls over the packed state (rows: l0 0:32, l1 32:48, l2 48:56,
  l3 56:72, l4 72:104; row 104 = ones for biases).
- PSUM per stream: bank0 = R | HN, bank1 = Z | N. zc = 1-z via negated
  z-weights. update: h' = h + zc*(n - h) as three fp16 DVE ops.

All tensors fp16 except PSUM (f32). Input XT / output YT are fp16
[64, T(+4), 128] per core; host transposes and converts.
"""

import numpy as np

D = 64
T_FULL = 512
BZ = 1024
NCORES = 8
BC = BZ // NCORES  # 128
H = [32, 16, 8, 16, 32]
OFFS = [0, 32, 48, 56, 72]
SH = 104
SA = 105

TAU = 8          # time segments per core
WARM = 20        # warmup ticks per segment
SEG = T_FULL // TAU  # 64
NSTREAM = 4      # streams; stream s carries segments (s, s+4)
GC = 2           # chains (segments) per stream
FREE = GC * BC   # 256
PW = 2 * FREE    # pair width: 512
NWIN = WARM + SEG + 4  # 88 windows per stream (wavefront drains 4 ticks)
NWC = 11         # windows per x chunk (NWIN % NWC == 0)
NRING = 16       # state ring depth (also h4 flush batching x2)
NFL = 8          # windows per h4 flush
YW0 = WARM + 4   # first window whose post-state holds a valid h4 tick
CH = 4096        # post-stage columns per chunk (32 ticks x 128 batch)
NCHUNK = T_FULL * BC // CH  # 16
CHT = CH // BC   # 32 ticks per chunk
POST = True        # emit the y-projection post-stage (off: timing probe only)
YF32 = False       # (dead end: DMA cannot source PSUM in this API)
TM_POOL = False    # n-gate product on GPSIMD instead of DVE
SIG_PAIR = True    # sigmoid granularity: per pair vs per stream
TANH_PAIR = True   # tanh granularity: per pair vs per stream


def _build_weights(inp):
    """Pack reference GRU weights into h-space wavefront matrices (fp16)."""
    f32 = np.float32
    WR = np.zeros((SA, SH), f32)
    WZ = np.zeros((SA, SH), f32)
    WN = np.zeros((SA, SH), f32)
    WHN = np.zeros((SA, SH), f32)
    W0 = np.zeros((D, 3 * SH), f32)
    for l in range(5):
        dh, o = H[l], OFFS[l]
        w_ih = np.asarray(inp[f"w_ih_{l}"], f32)
        w_hh = np.asarray(inp[f"w_hh_{l}"], f32)
        b_ih = np.asarray(inp[f"b_ih_{l}"], f32)
        b_hh = np.asarray(inp[f"b_hh_{l}"], f32)
        Wir, Wiz, Win = w_ih[:dh], w_ih[dh:2 * dh], w_ih[2 * dh:]
        Whr, Whz, Whn = w_hh[:dh], w_hh[dh:2 * dh], w_hh[2 * dh:]
        bir, biz, bin_ = b_ih[:dh], b_ih[dh:2 * dh], b_ih[2 * dh:]
        bhr, bhz, bhn = b_hh[:dh], b_hh[dh:2 * dh], b_hh[2 * dh:]
        WR[o:o + dh, o:o + dh] = Whr.T
        WZ[o:o + dh, o:o + dh] = -Whz.T
        WHN[o:o + dh, o:o + dh] = Whn.T
        WR[SH, o:o + dh] = bir + bhr
        WZ[SH, o:o + dh] = -(biz + bhz)
        WN[SH, o:o + dh] = bin_
        WHN[SH, o:o + dh] = bhn
        if l == 0:
            W0[:, 0:32] = Wir.T
            W0[:, SH:SH + 32] = -Wiz.T
            W0[:, 2 * SH:2 * SH + 32] = Win.T
        else:
            po, pd = OFFS[l - 1], H[l - 1]
            WR[po:po + pd, o:o + dh] = Wir.T
            WZ[po:po + pd, o:o + dh] = -Wiz.T
            WN[po:po + pd, o:o + dh] = Win.T
    w_out = np.asarray(inp["w_out"], f32)
    b_out = np.asarray(inp["b_out"], f32)
    # post-stage projection weight: rows 0:32 = w_out.T, row 32 = bias
    WY2 = np.zeros((33, D), f32)
    WY2[0:32, :] = w_out.T
    WY2[32, :] = b_out

    f16 = np.float16
    IDENT = np.eye(SH, dtype=f16)
    ZINIT = np.zeros((SA, PW), f16)
    ZINIT[SH] = 1.0
    RSTZ = np.zeros((32, BC), f16)
    return {"WR": WR.astype(f16), "WZ": WZ.astype(f16), "WN": WN.astype(f16),
            "WHN": WHN.astype(f16), "W0": W0.astype(f16),
            "WY2": WY2.astype(f16), "ZINIT": ZINIT, "RSTZ": RSTZ,
            "IDENT": IDENT}


def _split_excess_waits(nc, limit=1):
    """The walrus build here accepts at most one sync-wait per instruction;
    Tile emits several on barrier drains etc. Split extras onto NoOps."""
    from concourse import mybir

    n_new = 0
    for f in nc.m.functions:
        for bb in f.blocks:
            changed = False
            new_list = []
            for ins in bb.instructions:
                si = ins.sync_info
                if si is not None and si.on_wait and len(si.on_wait) > limit:
                    waits = list(si.on_wait)
                    while len(waits) > limit:
                        chunk, waits = waits[:limit], waits[limit:]
                        nop = mybir.InstNoOp(
                            name=f"{ins.name}-ws{n_new}",
                            engine=ins.engine,
                            sync_info=mybir.SyncInfo(on_wait=chunk, on_update=[]),
                        )
                        new_list.append(nop)
                        n_new += 1
                    ins.sync_info = mybir.SyncInfo(
                        on_wait=list(waits), on_update=list(si.on_update)
                    )
                    changed = True
                new_list.append(ins)
            if changed:
                bb.instructions = new_list
    return n_new


_prog_cache = {}


def _build_program(T, reps=1):
    key = (T, reps)
    if key in _prog_cache:
        return _prog_cache[key]
    assert T == T_FULL, "kernel is specialized for T=512"
    import concourse.bass as bass
    import concourse.tile as tile
    from concourse import mybir

    f16 = mybir.dt.float16
    f32 = mybir.dt.float32
    SIG = mybir.ActivationFunctionType.Sigmoid
    TANH = mybir.ActivationFunctionType.Tanh
    COPY = mybir.ActivationFunctionType.Copy

    TP = T + 4  # XT padded with 4 zero ticks for wavefront drain

    vtag = f"{int(POST)}{int(TM_POOL)}{int(SIG_PAIR)}{int(TANH_PAIR)}{int(YF32)}"
    nc = bass.Bass(trn_type="TRN2", name=f"gru_v3_{T}_{reps}_{vtag}")
    XT = nc.dram_tensor("XT", [D, TP, BC], f16, kind="ExternalInput")
    dWR = nc.dram_tensor("WR", [SA, SH], f16, kind="ExternalInput")
    dWZ = nc.dram_tensor("WZ", [SA, SH], f16, kind="ExternalInput")
    dWN = nc.dram_tensor("WN", [SA, SH], f16, kind="ExternalInput")
    dWHN = nc.dram_tensor("WHN", [SA, SH], f16, kind="ExternalInput")
    dW0 = nc.dram_tensor("W0", [D, 3 * SH], f16, kind="ExternalInput")
    dWY2 = nc.dram_tensor("WY2", [33, D], f16, kind="ExternalInput")
    dZINIT = nc.dram_tensor("ZINIT", [SA, PW], f16, kind="ExternalInput")
    dIDENT = nc.dram_tensor("IDENT", [SH, SH], f16, kind="ExternalInput")
    dRSTZ = nc.dram_tensor("RSTZ", [32, BC], f16, kind="ExternalInput")
    H4T = nc.dram_tensor("H4T", [32, T, BC], f16, kind="Internal")
    YT = nc.dram_tensor("YT", [D, T, BC], f32 if YF32 else f16,
                        kind="ExternalOutput")

    with tile.TileContext(nc) as tc:
        with (
            tc.tile_pool(name="consts", bufs=1) as consts,
            tc.tile_pool(name="xpool", bufs=2 * NSTREAM) as xpool,
            tc.tile_pool(name="work", bufs=2 * NSTREAM) as work,
            tc.tile_pool(name="ypool", bufs=3) as ypool,
            tc.tile_pool(name="ps", bufs=1, space="PSUM") as ps,
        ):
            wr = consts.tile([SA, SH], f16, tag="wr")
            wz = consts.tile([SA, SH], f16, tag="wz")
            wn = consts.tile([SA, SH], f16, tag="wn")
            whn = consts.tile([SA, SH], f16, tag="whn")
            w0 = consts.tile([D, 3 * SH], f16, tag="w0")
            wy2 = consts.tile([33, D], f16, tag="wy2")
            ident = consts.tile([SH, SH], f16, tag="ident")
            for i, (sb, dr) in enumerate(((wr, dWR), (wz, dWZ), (wn, dWN),
                                          (whn, dWHN), (w0, dW0),
                                          (wy2, dWY2), (ident, dIDENT))):
                eng = nc.sync if i % 2 == 0 else nc.gpsimd
                eng.dma_start(out=sb[:], in_=dr[:])

            # state rings: one per pair, 16 deep, pair width 512
            rings = [consts.tile([SA, NRING, PW], f16, tag=f"ring{p}",
                                  name=f"ring{p}") for p in range(2)]
            # post-stage input staging (33rd row = ones for the bias)
            h4s = [consts.tile([33, CH], f16, tag=f"h4s{j}",
                                name=f"h4s{j}") for j in range(2)]

            for _rep in range(reps):
                # --- init: ones rows in every ring slot; zero state in the
                # slot read by window 0 (slot NRING-1) ---
                for p in range(2):
                    nc.sync.dma_start(out=rings[p][:, NRING - 1, :],
                                      in_=dZINIT[:])
                for p in range(2):
                    # ones rows for slots 0..14: only needed from window
                    # sl+1 onward; keep them off the SP queue so the first
                    # x chunks aren't delayed
                    for sl in range(NRING - 1):
                        nc.gpsimd.dma_start(
                            out=rings[p][SH:SA, sl, :],
                            in_=dZINIT[SH:SA, :])
                # ones row for h4s: ZINIT row SH is ones but only PW wide;
                # fill via CH/PW copies
                if _rep == 0:
                    for j in range(2):
                        for q in range(CH // PW):
                            nc.gpsimd.dma_start(
                                out=h4s[j][32:33, q * PW:(q + 1) * PW],
                                in_=dZINIT[SH:SA, :])

                pTv = ps.tile([SH, 4, 2, 2, 256], f32, tag="pT", name="pT")
                xcs = [None] * NSTREAM

                def tA0(s):  # chain A (segment s) tick at window 0
                    return SEG * s - WARM

                def tB0(s):  # chain B (segment s+4) tick at window 0
                    return SEG * (s + 4) - WARM

                def load_xchunk(s, w0_):
                    nw = min(NWC, NWIN - w0_)
                    xc = xpool.tile([D, NWC, FREE], f16, tag="xc", name="xc")
                    eng = nc.sync if s % 2 == 0 else nc.gpsimd
                    for half, t0 in ((0, tA0(s) + w0_), (1, tB0(s) + w0_)):
                        lo = half * BC
                        if t0 >= 0:
                            eng.dma_start(
                                out=xc[:, 0:nw, lo:lo + BC],
                                in_=XT[:, t0:t0 + nw, :])
                        elif t0 + nw > 0:
                            k = -t0
                            eng.dma_start(
                                out=xc[:, k:nw, lo:lo + BC],
                                in_=XT[:, 0:nw - k, :])
                            eng.dma_start(
                                out=xc[:, 0:k, lo:lo + BC],
                                in_=XT[:, 0:k, :])  # garbage warmup pad
                        else:
                            eng.dma_start(
                                out=xc[:, 0:nw, lo:lo + BC],
                                in_=XT[:, 0:nw, :])  # garbage warmup pad
                    xcs[s] = xc

                def emit_xmm_rz(w):
                    """x prefetch matmuls for window w's R/Z quarters."""
                    for s in range(NSTREAM):
                        xw = xcs[s][:, w % NWC, :]
                        nc.tensor.matmul(pTv[:, s, 0, 0, :], w0[:, 0:SH],
                                         xw, start=True, stop=False)
                        nc.tensor.matmul(pTv[:, s, 1, 0, :],
                                         w0[:, SH:2 * SH], xw,
                                         start=True, stop=False)

                def window_all(w):
                    """One window for all streams. Chain-critical ops run per
                    stream; emission order matches expected readiness (each
                    engine executes its queue in order)."""
                    sl = w % NRING
                    pv = (w - 1) % NRING
                    xws, rzs, ntp = {}, {}, {}
                    if w % NWC == 0:
                        for s in range(NSTREAM):
                            load_xchunk(s, w)
                    for s in range(NSTREAM):
                        xws[s] = xcs[s][:, w % NWC, :]
                    if w % NWC == 0:
                        # chunk-boundary window: x matmuls were not
                        # prefetched at the previous window's tail
                        emit_xmm_rz(w)

                    def prev_ap(s):
                        p, js = s // 2, s % 2
                        return rings[p][:, pv, js * FREE:(js + 1) * FREE]

                    # PSUM per stream: bank0 = R | HN, bank1 = Z | N; group
                    # order per quarter: R: xmmR,mmR ; HN: mmHN ;
                    # Z: xmmZ,mmZ ; N: xmmN,mmN,fold
                    for s in range(NSTREAM):
                        prev = prev_ap(s)
                        nc.tensor.matmul(pTv[:, s, 0, 0, :], wr[:],
                                         prev, start=False, stop=True)
                        nc.tensor.matmul(pTv[:, s, 1, 0, :], wz[:],
                                         prev, start=False, stop=True)
                        nc.tensor.matmul(pTv[:, s, 0, 1, :], whn[:],
                                         prev, start=True, stop=True)
                    if SIG_PAIR:
                        for pair in range(2):
                            rz = work.tile([SH, 2, 2, 256], f16, tag="rz",
                                           name="rz")
                            nc.scalar.activation(
                                rz[:], pTv[:, 2 * pair:2 * pair + 2, :, 0, :],
                                SIG)
                            rzs[2 * pair] = rz[:, 0, :, :]
                            rzs[2 * pair + 1] = rz[:, 1, :, :]
                    else:
                        for s in range(NSTREAM):
                            rz = work.tile([SH, 2, 256], f16, tag="rz",
                                           name="rz")
                            nc.scalar.activation(rz[:], pTv[:, s, :, 0, :],
                                                 SIG)
                            rzs[s] = rz
                    # N-gate input matmuls
                    for s in range(NSTREAM):
                        nc.tensor.matmul(pTv[:, s, 1, 1, :],
                                         w0[:, 2 * SH:3 * SH], xws[s],
                                         start=True, stop=False)
                        nc.tensor.matmul(pTv[:, s, 1, 1, :], wn[:],
                                         prev_ap(s), start=False,
                                         stop=False)
                    # DVE: per-stream n-gate product; PE folds into pN
                    for s in range(NSTREAM):
                        tm = work.tile([SH, FREE], f16, tag="tm", name="tm")
                        eng = nc.gpsimd if TM_POOL else nc.vector
                        eng.tensor_mul(tm[:], rzs[s][:, 0, :],
                                       pTv[:, s, 0, 1, :])
                        nc.tensor.matmul(pTv[:, s, 1, 1, :], ident[:],
                                         tm[:], start=False, stop=True)
                    if TANH_PAIR:
                        for pair in range(2):
                            nt = work.tile([SH, 2, 256], f16, tag="nt",
                                           name="nt")
                            nc.scalar.activation(
                                nt[:],
                                pTv[:, 2 * pair:2 * pair + 2, 1, 1, :],
                                TANH)
                            ntp[2 * pair] = nt[:, 0, :]
                            ntp[2 * pair + 1] = nt[:, 1, :]
                    else:
                        for s in range(NSTREAM):
                            nt = work.tile([SH, FREE], f16, tag="nt",
                                           name="nt")
                            nc.scalar.activation(nt[:], pTv[:, s, 1, 1, :],
                                                 TANH)
                            ntp[s] = nt[:]
                    # per-stream state update into ring slot sl
                    for s in range(NSTREAM):
                        pair, js = s // 2, s % 2
                        prev = prev_ap(s)
                        zc_s = rzs[s][:, 1, :]
                        d = work.tile([SH, FREE], f16, tag="d", name="d")
                        nc.vector.tensor_sub(d[:], ntp[s], prev[0:SH, :])
                        p = work.tile([SH, FREE], f16, tag="p", name="p")
                        nc.vector.tensor_mul(p[:], d[:], zc_s)
                        gdst = rings[pair][0:SH, sl, js * FREE:(js + 1) * FREE]
                        nc.vector.tensor_add(gdst, p[:], prev[0:SH, :])
                        if s == 0:
                            # segment-0 warmup resets (chain A of stream 0)
                            if w == WARM - 1:
                                nc.sync.dma_start(
                                    out=rings[0][0:SH, sl, 0:BC],
                                    in_=dZINIT[0:SH, 0:BC])
                            elif WARM <= w < WARM + 4:
                                l = w - WARM + 1
                                nc.sync.dma_start(
                                    out=rings[0][OFFS[l]:OFFS[l] + H[l],
                                                 sl, 0:BC],
                                    in_=dRSTZ[0:H[l], :])
                    # prefetch next window's x matmuls (quarters are free
                    # once this window's sigmoid has read them); skip at
                    # chunk boundaries where the next chunk isn't loaded yet
                    if w + 1 < NWIN and (w + 1) % NWC != 0:
                        emit_xmm_rz(w + 1)
                    # h4 flush every NFL windows once ticks are valid
                    if w >= YW0 + NFL - 1 and (w - YW0) % NFL == NFL - 1:
                        k0 = w - YW0 - NFL + 1  # first tick of this flush
                        s0 = (w - NFL + 1) % NRING  # first ring slot
                        assert s0 + NFL <= NRING
                        for s in range(NSTREAM):
                            pair, js = s // 2, s % 2
                            for half, seg in ((0, s), (1, s + 4)):
                                t0 = SEG * seg + k0
                                lo = js * FREE + half * BC
                                nc.gpsimd.dma_start(
                                    out=H4T[:, t0:t0 + NFL, :],
                                    in_=rings[pair][OFFS[4]:OFFS[4] + 32,
                                                    s0:s0 + NFL,
                                                    lo:lo + BC])

                for w in range(NWIN):
                    window_all(w)

                # --- post-stage: y = WY2^T @ [h4; 1] over 16 chunks of
                # 4096 cols; the whole PSUM is one chunk, the f32->f16
                # narrow runs split across ACT and DVE in parallel ---
                for c in range(NCHUNK if POST else 0):
                    t0 = c * CHT
                    hb = h4s[c % 2]
                    nc.sync.dma_start(out=hb[0:32, :],
                                      in_=H4T[:, t0:t0 + CHT, :])
                    for q in range(8):
                        g = q // 2
                        b = q % 2
                        nc.tensor.matmul(
                            pTv[0:D, g, b, :, :],
                            wy2[:], hb[:, q * 512:(q + 1) * 512],
                            start=True, stop=True)
                    yo = ypool.tile([D, CH], f16, tag="yo", name="yo")
                    nc.scalar.activation(yo[:, 0:CH // 2],
                                         pTv[0:D, 0:2, :, :, :], COPY)
                    nc.vector.tensor_scalar_mul(yo[:, CH // 2:CH],
                                                pTv[0:D, 2:4, :, :, :], 1.0)
                    nc.gpsimd.dma_start(
                        out=YT[:, t0:t0 + CHT, :],
                        in_=yo[:].rearrange("p (t b) -> p t b", t=CHT))

    _split_excess_waits(nc)
    _prog_cache[key] = nc
    return nc


def _prep_inputs(X_full, weights, T):
    """X_full [BZ, T, D] fp32 -> per-core in_maps with fp16 padded XT."""
    maps = []
    for c in range(NCORES):
        xs = X_full[c * BC:(c + 1) * BC]  # [BC, T, D]
        xt = np.zeros((D, T + 4, BC), np.float16)
        xt[:, :T, :] = xs.transpose(2, 1, 0).astype(np.float16)
        maps.append({"XT": xt, **weights})
    return maps


def _run(X_full, weights, T):
    from concourse.bass_utils import run_bass_kernel_spmd

    nc = _build_program(T)
    in_maps = _prep_inputs(X_full, weights, T)
    res = run_bass_kernel_spmd(nc, in_maps, core_ids=list(range(NCORES)))
    outs = []
    for c in range(NCORES):
        YTc = res.results[c]["YT"]  # [D, T, BC] fp16
        outs.append(np.ascontiguousarray(
            YTc.astype(np.float32).transpose(2, 1, 0)))
    return np.concatenate(outs, 0)


def kernel(**inputs):
    X = np.asarray(inputs["imputed_X"], np.float32)
    weights = _build_weights(inputs)
    return _run(X, weights, X.shape[1])
